# revision 16
# baseline (speedup 1.0000x reference)
"""Trainium2 Bass kernel for nn_Assistance (colors_only path).

For each of 64x64=4096 patches (21x21 window, stride 2) of a 147x147x3
image: compute 3 wedge indicators from 5 params (ests), then the
wedge-weighted mean colors -> output (1, 3, 3, 64, 64).

Sharding: 8 cores x 8 patch rows (512 patches each). Each core gets its
35 relevant image rows + its ests shard, so the SPMD graph is
core-independent. No collectives.

Reformulation: with indicators h0, h1 and p01 = h0*h1,
  num_k,c = S_c - M1_c, M1_c - M2_c, M2_c ; wsum_k = 441 - H1, H1 - H2, H2
where S_c = sum(I_c), M1_c = sum(h0 I_c), M2_c = sum(p01 I_c),
H1 = sum(h0), H2 = sum(p01). The signed distances are evaluated with
the wedge sign FOLDED into the per-patch line coefficients, so each of
the 4 lines is exactly (X*nsg + dg) [gpsimd] then (Y*cg + .) [DVE stt].
Sums ride fused accum_out on the producing stt ops where possible.
"""
import os
import sys

for _p in ("/opt/trn_rl_repo", "/root/.axon_site/_ro/trn_rl_repo"):
    if os.path.isdir(_p) and _p not in sys.path:
        sys.path.insert(0, _p)

import numpy as np

import concourse.bass as bass
import concourse.bacc as bacc
import concourse.tile as tile
from concourse import mybir
from concourse.bass_utils import run_bass_kernel_spmd

F32 = mybir.dt.float32
OP = mybir.AluOpType
ACT = mybir.ActivationFunctionType

PI = float(np.pi)
R = 21
STRIDE = 2
ETA = 0.01
TAU = 0.1
H = W = 147
HP = WP = 64
NPIX = R * R  # 441
NCORES = 8
GROUPS = 4
ROWS_PER_CORE = 35


def _fit_sincos_coeffs():
    """LSQ poly coeffs for -sin(v), -cos(v) on [-pi, pi]; with
    v = (a mod 2pi) - pi these evaluate sin(a), cos(a) directly."""
    v = np.linspace(-PI, PI, 20001)
    A = np.stack([v ** (2 * k + 1) for k in range(7)], 1)
    cs = np.linalg.lstsq(A, -np.sin(v), rcond=None)[0]
    Ac = np.stack([v ** (2 * k) for k in range(8)], 1)
    cc = np.linalg.lstsq(Ac, -np.cos(v), rcond=None)[0]
    return [float(x) for x in cs], [float(x) for x in cc]


SIN_C, COS_C = _fit_sincos_coeffs()

# Engine assignment for the three per-group S_c reductions (tunable).
S_ENGINES = ("act", "act", "vector")


def build_nc():
    nc = bacc.Bacc()

    img_ext = nc.declare_dram_parameter("img", [ROWS_PER_CORE * W * 3], F32, isOutput=False)
    ests_ext = nc.declare_dram_parameter("ests", [512 * 5], F32, isOutput=False)
    xg_ext = nc.declare_dram_parameter("xg", [NPIX], F32, isOutput=False)
    yg_ext = nc.declare_dram_parameter("yg", [NPIX], F32, isOutput=False)
    id_ext = nc.declare_dram_parameter("ident", [128, 128], F32, isOutput=False)
    out_ext = nc.declare_dram_parameter("out", [9, 512], F32, isOutput=True)

    def bcast(ext, n):
        a = ext[:]
        return bass.AP(tensor=a.tensor, offset=a.offset, ap=[[0, 128], [1, n]])

    def dram_ap(ext, offset, dims):
        a = ext[:]
        return bass.AP(tensor=a.tensor, offset=a.offset + offset, ap=dims)

    with tile.TileContext(nc) as tc:
        with (
            tc.tile_pool(name="const", bufs=1) as const,
            tc.tile_pool(name="sc", bufs=1) as sc,
            tc.tile_pool(name="patch", bufs=2) as patchp,
            tc.tile_pool(name="work", bufs=2) as work,
            tc.tile_pool(name="scr", bufs=3) as scrp,
            tc.tile_pool(name="psum", bufs=1, space="PSUM") as psum,
        ):
            G = GROUPS

            # ---------------- constants ----------------
            X = const.tile([128, NPIX], F32)
            Y = const.tile([128, NPIX], F32)
            nc.sync.dma_start(out=X, in_=bcast(xg_ext, NPIX))
            nc.sync.dma_start(out=Y, in_=bcast(yg_ext, NPIX))
            ident = const.tile([128, 128], F32)
            nc.sync.dma_start(out=ident, in_=id_ext[:, :])
            HALF = const.tile([128, NPIX], F32)
            nc.gpsimd.memset(HALF, 0.5)

            # E[p, g, q] = ests[g*128 + p, q]
            E = const.tile([128, GROUPS, 5], F32)
            nc.sync.dma_start(
                out=E, in_=dram_ap(ests_ext, 0, [[5, 128], [128 * 5, GROUPS], [1, 5]])
            )

            # ---------------- helpers ----------------
            def mk(pool, shape, tag):
                return pool.tile(shape, F32, name=tag, tag=tag)

            def ts(eng, out, in0, s1, s2=None, op0=OP.mult, op1=OP.add):
                if s2 is None:
                    eng.tensor_scalar(out=out, in0=in0, scalar1=s1, scalar2=None, op0=op0)
                else:
                    eng.tensor_scalar(out=out, in0=in0, scalar1=s1, scalar2=s2, op0=op0, op1=op1)

            def stt(out, in0, s, in1, op0, op1, accum_out=None):
                kw = {"accum_out": accum_out} if accum_out is not None else {}
                nc.vector.scalar_tensor_tensor(
                    out=out, in0=in0, scalar=s, in1=in1, op0=op0, op1=op1, **kw
                )

            def tt(eng, out, a, b, op):
                eng.tensor_tensor(out=out, in0=a, in1=b, op=op)

            V, GP = nc.vector, nc.gpsimd

            def t4(tag, pool=sc):
                return mk(pool, [128, G], tag)

            # ------------- per-patch scalars -------------
            # thetas for q=0..2 batched: T12 [128, G, 3]
            E3 = E[:, :, 0:3]
            T12 = mk(sc, [128, G, 3], "T12")
            T12F = T12[:, :, :].rearrange("p g q -> p (g q)")
            ts(GP, T12[:, :, :], E3, PI, PI)   # (e+1)*pi
            # floor((e+1)/2) via compares against j*2pi, j in {-1,0,1,2}; base -2
            cacc = None
            for i, j in enumerate((-1.0, 0.0, 1.0, 2.0)):
                cj = mk(sc, [128, G * 3], f"cj{i % 2}")
                ts(GP, cj, T12F, j * 2 * PI, None, OP.is_ge)
                if cacc is None:
                    cacc = cj
                else:
                    nxt = mk(sc, [128, G * 3], f"ca{i % 2}")
                    tt(GP, nxt, cacc, cj, OP.add)
                    cacc = nxt
            mterm = mk(sc, [128, G * 3], "mterm")
            ts(GP, mterm, cacc, -2 * PI, 4 * PI)                      # -2pi*(sum) + 4pi
            TH = mk(sc, [128, G, 3], "TH")
            tt(GP, TH[:, :, :].rearrange("p g q -> p (g q)"), T12F, mterm, OP.add)

            th = [TH[:, :, j] for j in range(3)]
            a1 = t4("a1"); a3 = t4("a3"); a2 = t4("a2")
            tmp = t4("tmp"); tmp2 = t4("tmp2")
            tt(V, tmp, th[0], th[1], OP.min); tt(V, a1, tmp, th[2], OP.min)
            tt(V, tmp2, th[0], th[1], OP.max); tt(V, a3, tmp2, th[2], OP.max)
            ssum = t4("ssum"); ssum2 = t4("ssum2"); s_m1 = t4("s_m1")
            tt(GP, ssum, th[0], th[1], OP.add)
            tt(GP, ssum2, ssum, th[2], OP.add)
            tt(V, s_m1, ssum2, a1, OP.subtract)
            tt(V, a2, s_m1, a3, OP.subtract)

            x0 = t4("x0"); y0 = t4("y0")
            ts(GP, x0, E[:, :, 3], 3.0, None)
            ts(GP, y0, E[:, :, 4], 3.0, None)

            # a4 = 0.5*(a1+a3) + pi*[mod(0.5*(a1-a3), 2pi) >= pi]
            df = t4("df"); nh = t4("nh")
            ts(V, nh, a3, -0.5, None)
            stt(df, a1, 0.5, nh, OP.mult, OP.add)
            c0 = t4("c0"); gp2 = t4("gp2"); m4 = t4("m4"); ge = t4("ge")
            ts(V, c0, df, 0.0, None, OP.is_lt)
            ts(V, gp2, c0, 2 * PI, None)
            tt(V, m4, df, gp2, OP.add)
            ts(V, ge, m4, PI, None, OP.is_ge)
            gpi = t4("gpi")
            ts(V, gpi, ge, PI, None)
            a4h = t4("a4h")
            stt(a4h, a3, 0.5, gpi, OP.mult, OP.add)
            a4 = t4("a4")
            stt(a4, a1, 0.5, a4h, OP.mult, OP.add)

            # D13 = a3 - a1; D42 = mod(a2 - a4, 2pi)
            D13 = t4("D13")
            tt(V, D13, a3, a1, OP.subtract)
            dd = t4("dd")
            tt(V, dd, a2, a4, OP.subtract)                   # in (-3pi, 2pi)
            ca = t4("ca"); cb = t4("cb"); cs_ = t4("cs_"); mt = t4("mt")
            ts(GP, ca, dd, -2 * PI, None, OP.is_ge)
            ts(GP, cb, dd, 0.0, None, OP.is_ge)
            tt(GP, cs_, ca, cb, OP.add)
            ts(GP, mt, cs_, -2 * PI, 4 * PI)
            D42 = t4("D42")
            tt(GP, D42, dd, mt, OP.add)

            # sgn = 2*[D < pi] - 1
            sgn13 = t4("sgn13"); sgn42 = t4("sgn42")
            cl = t4("cl"); cl2 = t4("cl2")
            ts(V, cl, D13, PI, None, OP.is_lt)
            ts(V, sgn13, cl, 2.0, -1.0)
            ts(GP, cl2, D42, PI, None, OP.is_lt)
            ts(GP, sgn42, cl2, 2.0, -1.0)

            # gt = tau * (D/pi - 1)^35
            def pow35(src, tag, eng):
                v = t4(tag + "v")
                ts(eng, v, src, 1.0 / PI, -1.0)
                v2 = t4(tag + "2"); tt(eng, v2, v, v, OP.mult)
                v3 = t4(tag + "3"); tt(eng, v3, v2, v, OP.mult)
                v4 = t4(tag + "4"); tt(eng, v4, v2, v2, OP.mult)
                v8 = t4(tag + "8"); tt(eng, v8, v4, v4, OP.mult)
                v16 = t4(tag + "16"); tt(eng, v16, v8, v8, OP.mult)
                v32 = t4(tag + "32"); tt(eng, v32, v16, v16, OP.mult)
                v35 = t4(tag + "35"); tt(eng, v35, v32, v3, OP.mult)
                gt = t4(tag + "gt")
                ts(eng, gt, v35, TAU, None)
                return gt

            gt13 = pow35(D13, "g13", V)
            gt42 = pow35(D42, "g42", GP)

            # ---- sin/cos of a1..a4 stacked [128, 4, G] ----
            A = mk(sc, [128, 4, G], "angles")
            for i, a in enumerate((a1, a2, a3, a4)):
                nc.gpsimd.tensor_copy(out=A[:, i, :], in_=a)
            AF = A[:, :, :].rearrange("p a g -> p (a g)")

            def t16(tag):
                return mk(sc, [128, 4 * G], tag)

            uw = t16("uw")
            ts(GP, uw, AF, 2 * PI, None, OP.is_ge)
            ar = t16("ar")
            # a' - pi = a - 2pi*u - pi : mt2 = -2pi*u - pi then add a
            mt2 = t16("mt2")
            ts(GP, mt2, uw, -2 * PI, -PI)
            tt(GP, ar, AF, mt2, OP.add)
            v2t = t16("v2t")
            tt(GP, v2t, ar, ar, OP.mult)
            # sin poly (DVE), cos poly (gpsimd)
            ps = t16("psa")
            ts(V, ps, v2t, SIN_C[6], SIN_C[5])
            for k in range(4, -1, -1):
                q = t16("psq" + ("a" if k % 2 else "b"))
                tt(V, q, ps, v2t, OP.mult)
                psn = t16("ps" + ("a" if k % 2 else "b"))
                ts(V, psn, q, SIN_C[k], None, OP.add)
                ps = psn
            SIN = mk(sc, [128, 4, G], "SIN")
            tt(V, SIN[:, :, :].rearrange("p a g -> p (a g)"), ps, ar, OP.mult)
            pc = t16("pca")
            ts(GP, pc, v2t, COS_C[7], COS_C[6])
            for k in range(5, -1, -1):
                qc = t16("pcq" + ("a" if k % 2 else "b"))
                tt(GP, qc, pc, v2t, OP.mult)
                pcn = t16("pc" + ("a" if k % 2 else "b"))
                ts(GP, pcn, qc, COS_C[k], None, OP.add)
                pc = pcn
            COS = mk(sc, [128, 4, G], "COS")
            nc.gpsimd.tensor_copy(out=COS[:, :, :].rearrange("p a g -> p (a g)"), in_=pc)

            # d_a = sin*x0 - cos*y0 per angle
            DAv = mk(sc, [128, 4, G], "DAv")
            for i in range(4):
                sx = t4(f"sx{i}")
                tt(V, sx, SIN[:, i, :], x0, OP.mult)
                cy = t4(f"cy{i}")
                tt(GP, cy, COS[:, i, :], y0, OP.mult)
                tt(V, DAv[:, i, :], sx, cy, OP.subtract)

            # ---- role-folded coefficients ----
            # roles: 0:(a4,+sgn42) 1:(a2,-sgn42) 2:(a1,+sgn13) 3:(a3,-sgn13)
            # line_r = X*nsg_r + Y*cg_r + dg_r where nsg = -sin*s, cg = cos*s, dg = d*s
            nsg42 = t4("nsg42"); nsg13 = t4("nsg13")
            ts(V, nsg42, sgn42, -1.0, None)
            ts(V, nsg13, sgn13, -1.0, None)
            ROLES = [(3, sgn42), (1, nsg42), (0, sgn13), (2, nsg13)]
            NSG = mk(sc, [128, 4, G], "NSG")
            CG = mk(sc, [128, 4, G], "CG")
            DG = mk(sc, [128, 4, G], "DG")
            for r, (ai, sg) in enumerate(ROLES):
                nsx = t4(f"nsx{r}")
                tt(V, nsx, SIN[:, ai, :], sg, OP.mult)       # sin*s
                ts(V, NSG[:, r, :], nsx, -1.0, None)         # -sin*s
                tt(GP, CG[:, r, :], COS[:, ai, :], sg, OP.mult)
                tt(V, DG[:, r, :], DAv[:, ai, :], sg, OP.mult)

            # ---- accumulators ----
            # R_dve: M1 @ c*4+g (0..11), M2 @ 16+c*4+g, H1 @ 48+g, H2 @ 52+g,
            #        S(dve-assigned) @ 32+c*4+g
            # R_act: S(act-assigned) @ c*4+g
            R_dve = const.tile([128, 64], F32)
            R_act = const.tile([128, 64], F32)

            # ---------------- main loop ----------------
            for g in range(GROUPS):
                patch = mk(patchp, [128, R, 63], "patch")
                for dh in range(2):
                    row0 = 4 * g + 2 * dh
                    nc.sync.dma_start(
                        out=patch[dh * 64:(dh + 1) * 64, :, :],
                        in_=dram_ap(img_ext, row0 * W * 3,
                                    [[STRIDE * 3, 64], [W * 3, R], [1, 63]]),
                    )

                # lines (sign-folded): gpsimd X part, DVE Y part fused with add
                Ls = []
                for r in range(4):
                    px = mk(work, [128, NPIX], f"px{r}")
                    GP.tensor_scalar(out=px, in0=X, scalar1=NSG[:, r, g:g + 1],
                                     scalar2=DG[:, r, g:g + 1], op0=OP.mult, op1=OP.add)
                    ln = mk(work, [128, NPIX], f"ln{r}")
                    stt(ln, Y, CG[:, r, g:g + 1], px, OP.mult, OP.add)
                    Ls.append(ln)

                D = mk(work, [128, 2 * NPIX], "D")
                mn42 = mk(work, [128, NPIX], "mn42")
                tt(V, mn42, Ls[0], Ls[1], OP.min)
                mn13 = mk(work, [128, NPIX], "mn13")
                tt(V, mn13, Ls[2], Ls[3], OP.min)
                ts(V, D[:, 0:NPIX], mn13, sgn13[:, g:g + 1], gt13[:, g:g + 1])
                ts(V, D[:, NPIX:], mn42, sgn42[:, g:g + 1], gt42[:, g:g + 1])

                T = mk(work, [128, 2 * NPIX], "T")
                nc.scalar.activation(out=T, in_=D, func=ACT.Arctan, scale=1.0 / ETA)

                h0 = mk(work, [128, NPIX], "h0")
                stt(h0, T[:, 0:NPIX], 1.0 / PI, HALF, OP.mult, OP.add,
                    accum_out=R_dve[:, 48 + g:49 + g])
                h1 = mk(work, [128, NPIX], "h1")
                GP.tensor_scalar(out=h1, in0=T[:, NPIX:], scalar1=1.0 / PI,
                                 scalar2=0.5, op0=OP.mult, op1=OP.add)
                p01 = mk(work, [128, NPIX], "p01")
                stt(p01, h0, 1.0, h1, OP.mult, OP.mult,
                    accum_out=R_dve[:, 52 + g:53 + g])

                for c in range(3):
                    Ic = patch[:, :, c::3]
                    col = c * 4 + g
                    m1o = mk(scrp, [128, R, R], "m1o")
                    stt(m1o, h0[:, :].rearrange("p (r s) -> p r s", r=R),
                        1.0, Ic, OP.mult, OP.mult,
                        accum_out=R_dve[:, col:col + 1])
                    m2o = mk(scrp, [128, R, R], "m2o")
                    stt(m2o, p01[:, :].rearrange("p (r s) -> p r s", r=R),
                        1.0, Ic, OP.mult, OP.mult,
                        accum_out=R_dve[:, col + 16:col + 17])
                    if S_ENGINES[c] == "act":
                        so = mk(scrp, [128, R, R], "so")
                        nc.scalar.activation(out=so, in_=Ic, func=ACT.Copy,
                                             accum_out=R_act[:, col:col + 1])
                    else:
                        nc.vector.tensor_reduce(
                            out=R_dve[:, col + 32:col + 33], in_=Ic,
                            axis=mybir.AxisListType.XY, op=OP.add)

            # ---------------- epilogue ----------------
            Sparts = []
            for c in range(3):
                src = R_act if S_ENGINES[c] == "act" else R_dve
                off = c * 4 if S_ENGINES[c] == "act" else 32 + c * 4
                Sparts.append(src[:, off:off + 4])
            M1v = R_dve[:, 0:12].rearrange("p (c g) -> p c g", c=3)
            M2v = R_dve[:, 16:28].rearrange("p (c g) -> p c g", c=3)
            H1v = R_dve[:, 48:52]
            H2v = R_dve[:, 52:56]

            C = const.tile([128, 3, 3, GROUPS], F32)  # (c, k, g)
            for c in range(3):
                tt(V, C[:, c, 0, :], Sparts[c], M1v[:, c, :], OP.subtract)
            tt(V, C[:, :, 1, :], M1v, M2v, OP.subtract)
            nc.gpsimd.tensor_copy(out=C[:, :, 2, :], in_=M2v)

            Wt = const.tile([128, 3, GROUPS], F32)  # (k, g)
            ts(V, Wt[:, 0, :], H1v, -1.0, float(NPIX))
            tt(V, Wt[:, 1, :], H1v, H2v, OP.subtract)
            nc.gpsimd.tensor_copy(out=Wt[:, 2, :], in_=H2v)

            W2 = const.tile([128, 3 * GROUPS], F32)
            ts(V, W2, Wt[:, :, :].rearrange("p k g -> p (k g)"), 1e-10, None, OP.add)
            VW = const.tile([128, 3, GROUPS], F32)
            nc.vector.reciprocal(out=VW[:, :, :].rearrange("p k g -> p (k g)"), in_=W2)

            C2 = const.tile([128, 3, 3, GROUPS], F32)
            for c in range(3):
                tt(V, C2[:, c, :, :], C[:, c, :, :], VW[:, :, :], OP.mult)

            pt = psum.tile([36, 128], F32)
            nc.tensor.transpose(
                pt[:, :], C2[:, :, :, :].rearrange("p a b c -> p (a b c)"), ident[:, :]
            )
            Tout = const.tile([36, 128], F32)
            nc.scalar.copy(out=Tout, in_=pt)
            nc.sync.dma_start(
                out=out_ext[:, :].rearrange("a (b c) -> a b c", b=GROUPS),
                in_=Tout,
            )

    nc.finalize()
    return nc


_NC_CACHE = None


def _get_nc():
    global _NC_CACHE
    if _NC_CACHE is None:
        _NC_CACHE = build_nc()
    return _NC_CACHE


def make_in_maps(ests, noisy_image):
    img = np.ascontiguousarray(np.asarray(noisy_image, dtype=np.float32)[0])
    ests = np.asarray(ests, dtype=np.float32).reshape(HP * WP, 5)
    grid = np.linspace(-1.0, 1.0, R, dtype=np.float32)
    xg = np.tile(grid, R)
    yg = np.repeat(grid, R)
    ident = np.eye(128, dtype=np.float32)
    in_maps = []
    for m in range(NCORES):
        in_maps.append({
            "img": np.ascontiguousarray(img[16 * m:16 * m + ROWS_PER_CORE]).reshape(-1),
            "ests": np.ascontiguousarray(ests[m * 512:(m + 1) * 512]).reshape(-1),
            "xg": xg, "yg": yg, "ident": ident,
        })
    return in_maps


def assemble(results):
    out = np.empty((1, 3, 3, HP, WP), np.float32)
    for m in range(NCORES):
        out[0, :, :, 8 * m:8 * m + 8, :] = results[m]["out"].reshape(3, 3, 8, WP)
    return out


def kernel(ests, noisy_image, gt_image=None, alpha=None, **_):
    nc = _get_nc()
    in_maps = make_in_maps(ests, noisy_image)
    res = run_bass_kernel_spmd(nc, in_maps, core_ids=list(range(NCORES)))
    return assemble(res.results)
